# revision 68
# baseline (speedup 1.0000x reference)
"""DySepConvAtten Trainium2 kernel (v4).

out = LayerNorm( pw @ relu(depthwise_conv1d(value, dw)) ), where
[dw | pw] = query @ W_wl + b_wl  per (batch, position).

Sharding: pure data parallelism, B=512 split over 8 NeuronCores (64 each).

v4 design (vs v3): merged-latch conv.
The K=3 depthwise conv runs as TWO custom DVE ops per sub-slab of 4
batches (instead of 8 per-batch ops).  Each op streams [100, 4, 257]
pages; the per-batch conv scale is carried as the page's first stream
element and captured into the DVE swap flop by a latch-init micro-op.
The uop FSM is hand-patched so SUB_DIM_DONE re-enters the latch state,
re-latching the scale at every page (batch) boundary:

  A buffer per batch: [dw0, z, v0..v255]   (z: 0 during op1, dw1 after)
  B buffer per batch: [dw2, v1..v255, 0]   (built on-chip by SBUF DMA)
  op1  acc[c]  = dw0*v[c-1] + dw2*v[c+1]     (dual latch, pages of 257)
  op2  depth[c]= relu(dw1*v[c] + acc[c])     (single latch + streamed acc)

LN stats via bn_stats/bn_aggr per batch (fp32, on DVE); normalize and
pwT bias on ScalarE; DMA triggers on Sync.
"""

import numpy as np

B, N, C, K = 512, 100, 256, 3
NCORES = 8
NB = B // NCORES          # batches per core
SLAB = 8                  # batches per slab
WARM = 2                  # leading slabs with host-precomputed dw/pwT
LN_EPS = 1e-5
SS = 4                    # compute sub-slab (batches); SLAB must be 2*SS

_cache: dict = {}
_ops_registered = [False]


def _mconv1_ref(in0, in1, s0, s1, imm2):
    # in0: [P, G, 257] (latched); in1: [P, G, 256]; out: [P, G, 256]
    f = np.float32
    L0 = in0[:, :, 0:1].astype(f)
    return (L0 * in0[:, :, 1:].astype(f) + in1.astype(f)).astype(f)


def _mconv2_ref(in0, in1, s0, s1, imm2):
    # in0/in1: [P, G, 257] (both latched); out: [P, G, 256]
    f = np.float32
    L0 = in0[:, :, 0:1].astype(f)
    L1 = in1[:, :, 0:1].astype(f)
    return np.maximum(L0 * in0[:, :, 1:].astype(f)
                      + L1 * in1[:, :, 1:].astype(f), 0.0).astype(f)


def _register_custom_ops():
    """Merged-latch conv ops with per-page (per-batch) scale re-latch."""
    if _ops_registered[0]:
        return
    import copy
    from concourse import dve_ops
    from concourse.dve_spec import (Spec, Src0, Src1, Latch, relu, lower,
                                    _has_src1)
    from concourse.dve_uop import DveOpSpec, Trigger

    if any(o.name == "ANT_MCONV1" for o in dve_ops.OPS):
        _ops_registered[0] = True
        return

    def patch_fsm(uops):
        # lower() gives [latch_init, steady].  Rebuild as:
        #   0: latch entry  (COUNT(1) -> 2)
        #   1: re-latch     (COUNT(1) -> 2)   <- SUB_DIM_DONE target
        #   2: steady       (SRC_DONE -> idle, SUB_DIM_DONE -> 1)
        assert len(uops) == 2
        L_entry, S = uops
        L_re = copy.deepcopy(L_entry)
        L_entry.next_uop = (2, 0, 0)
        L_re.next_uop = (2, 0, 0)
        S.trigger = (Trigger.SRC_TENSOR_DONE, Trigger.SUB_DIM_DONE,
                     Trigger.NONE)
        S.next_uop = (0, 1, 0)
        return [L_entry, L_re, S]

    def make(name, spec, next_row):
        # Build patched specs for both hw generations and seed the
        # compile cache so DveOp.compile() returns the patched FSM
        # (its own lower() call would lose the patch).
        shas = {}
        for ver in ("v3", "v4"):
            uops = patch_fsm(lower(spec, ver=ver))
            for u in uops:
                u.validate(ver)
            s = DveOpSpec(name=name, opcode=next_row, uops=uops,
                          rd1_en=_has_src1(spec))
            shas[ver] = s.sha(ver)
            dve_ops._COMPILE_CACHE[(name, ver)] = s
        return dve_ops.DveOp(name, spec, subdim=True, uops_sha=shas)

    specs = [
        ("ANT_MCONV1", Spec(
            body=Latch(Src0) * Src0 + Src1,
            reference=_mconv1_ref)),
        ("ANT_MCONV2", Spec(
            body=relu(Latch(Src0) * Src0 + Latch(Src1) * Src1),
            reference=_mconv2_ref)),
    ]
    for name, spec in specs:
        row = dve_ops._CUSTOM_DVE_ROW_BASE + len(dve_ops.OPS)
        op = make(name, spec, row)
        dve_ops.OPS.append(op)
        dve_ops._SUB_OPCODE_FOR_NAME[name] = row
        dve_ops.CUSTOM_DVE_SPECS[name] = spec
        setattr(dve_ops, name, op)
    _ops_registered[0] = True


def _build(nb: int):
    import concourse.bass as bass
    import concourse.tile as tile
    from concourse import bacc, mybir
    from concourse import dve_ops

    _register_custom_ops()
    MCONV1 = dve_ops.ANT_MCONV1
    MCONV2 = dve_ops.ANT_MCONV2

    fp32 = mybir.dt.float32
    bf16 = mybir.dt.bfloat16
    AF = mybir.ActivationFunctionType
    OP = mybir.AluOpType

    nc = bacc.Bacc("TRN2", target_bir_lowering=False, debug=False)

    nsl = nb // SLAB
    warm = min(WARM, nsl)

    qT_d = nc.dram_tensor("qT", (nsl - warm, 128, SLAB, 2 * N), bf16,
                          kind="ExternalInput")
    # v layout per batch: [r0-slot, 0, v0..v255, 0, dw2-slot] (260 cols)
    v_d = nc.dram_tensor("v", (nsl, N, SLAB, C + 4), bf16, kind="ExternalInput")
    w2pw_d = nc.dram_tensor("w2pw", (128, 2, N), bf16, kind="ExternalInput")
    w2dw_d = nc.dram_tensor("w2dw", (128, 2, K), bf16, kind="ExternalInput")
    bpw_d = nc.dram_tensor("bpw", (N, 1), fp32, kind="ExternalInput")
    bdwb_d = nc.dram_tensor("bdwb", (N, SLAB, K), fp32, kind="ExternalInput")
    dw0_d = nc.dram_tensor("dw0", (N, warm * SLAB, K), fp32, kind="ExternalInput")
    pwT0_d = nc.dram_tensor("pwT0", (N, warm, SLAB * N), bf16, kind="ExternalInput")
    out_d = nc.dram_tensor("out", (nsl, N, SLAB, C), bf16, kind="ExternalOutput")

    with tile.TileContext(nc) as tc:
        with (
            tc.tile_pool(name="const", bufs=1) as cpool,
            tc.tile_pool(name="slab_in", bufs=4) as sin_pool,
            tc.tile_pool(name="slab_out", bufs=3) as sout_pool,
            tc.tile_pool(name="work", bufs=6) as wpool,
            tc.tile_pool(name="pwt", bufs=8) as ppool,
            tc.tile_pool(name="small", bufs=6) as spool,
            tc.tile_pool(name="ps_pwT", bufs=1, space="PSUM") as ps_pwT_pool,
            tc.tile_pool(name="ps_dw", bufs=1, space="PSUM") as ps_dw_pool,
            tc.tile_pool(name="ps_pair", bufs=6, space="PSUM") as ps_pair_pool,
        ):
            dw_sb0 = cpool.tile([N, warm * SLAB, K], fp32)
            pwT_sb0 = cpool.tile([N, warm, SLAB * N], bf16)
            w2pw_t = cpool.tile([128, 2, N], bf16)
            w2dw_t = cpool.tile([128, 2, K], bf16)
            bpw_t = cpool.tile([N, 1], fp32)
            bdwb_t = cpool.tile([N, SLAB, K], fp32)
            eps_t = cpool.tile([N, 1], fp32)
            nc.gpsimd.memset(eps_t[:], LN_EPS)
            zc = cpool.tile([N, SLAB, 1], fp32)
            nc.gpsimd.memset(zc[:], 0.0)
            # touch Sqrt+Identity early so the ACT table load overlaps
            # the initial DMAs instead of stalling the first sub-slab
            warm_act = cpool.tile([N, 2], fp32)
            nc.scalar.activation(warm_act[:, 0:1], eps_t[:], AF.Sqrt)
            nc.scalar.activation(warm_act[:, 1:2], eps_t[:], AF.Identity)

            state = {}
            nss = nsl * 2          # sub-slabs of SS batches

            def stage0(s):
                """dy-chain + conv head prep for sub-slab s (runs a
                sub-slab ahead of the conv so the cross-engine head
                writes never stall the DVE queue)."""
                d, h = s // 2, s % 2
                A = state[("v", d)]
                if d < warm:
                    dw_sb = dw_sb0[:, s * SS:(s + 1) * SS, :]
                    pwT_sb = pwT_sb0[:, d, h * SS * N:(h + 1) * SS * N]
                else:
                    qT_s = state[("q", d)]
                    if h == 1:
                        del state[("q", d)]
                    qs = qT_s[:, h * SS:(h + 1) * SS, :]
                    # pwT: out[m, (j,n)] = sum_c W[c, K+m] qT[c, (j,n)]
                    ps_pwT = ps_pwT_pool.tile([N, 512], fp32, tag="ps_pwT")
                    nc.tensor.matmul(ps_pwT[:, 0:SS * N], w2pw_t[:, 0, :],
                                     qs[:, :, 0:N], start=True, stop=False)
                    nc.tensor.matmul(ps_pwT[:, 0:SS * N], w2pw_t[:, 1, :],
                                     qs[:, :, N:2 * N], start=False, stop=True)
                    pwT_sb = ppool.tile([N, SS * N], bf16, tag="pwT_sb")
                    nc.scalar.activation(pwT_sb[:], ps_pwT[:, 0:SS * N],
                                         AF.Identity, bias=bpw_t[:])
                    # dw: per batch, qT slice as stationary
                    ps_dw = ps_dw_pool.tile([N, SS, K], fp32, tag="ps_dw")
                    for j in range(SS):
                        nc.tensor.matmul(ps_dw[:, j, :], qs[:, j, 0:N],
                                         w2dw_t[:, 0, :], start=True, stop=False)
                        nc.tensor.matmul(ps_dw[:, j, :], qs[:, j, N:2 * N],
                                         w2dw_t[:, 1, :], start=False, stop=True)
                    dw_sb = spool.tile([N, SS, K], fp32, tag="dw_sb")
                    # demote this DVE op in the scheduler's order: its
                    # dep chain runs through qT-DMA and the PE queue and
                    # it otherwise head-of-line blocks ready convs
                    with tc.high_priority(offset=-45):
                        nc.vector.tensor_tensor(dw_sb[:], ps_dw[:],
                                                bdwb_t[:, 0:SS, :], op=OP.add)

                j0 = h * SS
                Av = A[:, j0:j0 + SS, :]
                # r0 = dw0/dw1; heads: r0 -> A col0, dw2 -> A col259,
                # dw1 -> U col256 (before op1 — disjoint from op1's out).
                # Head copies ride GpSimd as TT-adds with a zero column.
                # Warm slabs: host prefilled r0/dw2; dw1 copy on the
                # then-idle DVE.
                U = wpool.tile([N, SS, C + 1], bf16, tag="u_t")
                if d < warm:
                    nc.vector.tensor_copy(U[:, :, C:C + 1], dw_sb[:, :, 1:2])
                else:
                    rec = spool.tile([N, SS, 1], fp32, tag="rec")
                    with tc.high_priority(offset=-45):
                        nc.vector.reciprocal(rec[:], dw_sb[:, :, 1:2])
                    nc.gpsimd.tensor_tensor(Av[:, :, 0:1], dw_sb[:, :, 0:1],
                                            rec[:], op=OP.mult)
                    nc.scalar.activation(Av[:, :, C + 3:C + 4],
                                         dw_sb[:, :, 2:3], AF.Identity)
                    nc.scalar.activation(U[:, :, C:C + 1], dw_sb[:, :, 1:2],
                                         AF.Identity)
                state[("s0", s)] = (pwT_sb, U)

            def stage1(s):
                """merged-latch conv for sub-slab s."""
                d, h = s // 2, s % 2
                A = state[("v", d)]
                if h == 1:
                    del state[("v", d)]
                pwT_sb, U = state.pop(("s0", s))
                j0 = h * SS
                Av = A[:, j0:j0 + SS, :]
                # op1 (fwd): u[c] = r0*v[c-1] + v[c]
                nc.vector._custom_dve(
                    MCONV1, out=U[:, :, 0:C],
                    in0=Av[:, :, 0:C + 1], in1=Av[:, :, 2:C + 2])
                # op2 (bwd): depth[c] = relu(dw1*u[c] + dw2*v[c+1])
                depth_s = wpool.tile([N, SS, C], bf16, tag="depth_s")
                nc.vector._custom_dve(
                    MCONV2, out=depth_s[:, :, C - 1::-1],
                    in0=U[:, :, C::-1], in1=Av[:, :, C + 3:2:-1])
                state[("s1", s)] = (pwT_sb, depth_s)

            def stage2a(s):
                """pointwise matmul + LN stats for sub-slab s."""
                pwT_sb, depth_s = state.pop(("s1", s))
                pairs = []
                for p in range(SS // 2):
                    pair = ps_pair_pool.tile([N, 2, C], fp32, tag="pair")
                    pairs.append(pair)
                for j in range(SS):
                    pw_j = pwT_sb[:, j * N:(j + 1) * N]
                    nc.tensor.matmul(pairs[j // 2][:, j % 2, :], pw_j,
                                     depth_s[:, j, :], start=True, stop=True)
                # LN stats via bn_stats per batch (the BIR verifier pins
                # bn_stats/bn_aggr to one group per instruction).
                st = spool.tile([N, SS, 6], fp32, tag="st")
                for j in range(SS):
                    nc.vector.bn_stats(st[:, j, :], pairs[j // 2][:, j % 2, :])
                sa = spool.tile([N, SS, 3], fp32, tag="sa")
                for j in range(SS):
                    nc.vector.bn_aggr(sa[:, j, 0:2], st[:, j, :])
                # rs = 1/sqrt(var + eps) ; nmr = -mean*rs
                std = spool.tile([N, SS], fp32, tag="std")
                nc.scalar.activation(std[:], sa[:, :, 1], AF.Sqrt,
                                     bias=eps_t[:])
                rs = spool.tile([N, SS], fp32, tag="rs")
                nc.vector.reciprocal(rs[:], std[:])
                nmr = spool.tile([N, SS], fp32, tag="nmr")
                nc.vector.scalar_tensor_tensor(nmr[:], sa[:, :, 0], -1.0, rs[:],
                                               op0=OP.mult, op1=OP.mult)
                state[("s2a", s)] = (pairs, rs, nmr)

            def stage2b(s):
                """normalize + store for sub-slab s."""
                d, h = s // 2, s % 2
                pairs, rs, nmr = state.pop(("s2a", s))
                if h == 0:
                    out_s = sout_pool.tile([N, SLAB, C], bf16, tag="out_s")
                    state[("o", d)] = out_s
                else:
                    out_s = state[("o", d)]
                norm_dve = (1, 3) if s >= nss - 2 else ()
                for j in range(SS):
                    ps_j = pairs[j // 2][:, j % 2, :]
                    oj = out_s[:, h * SS + j, :]
                    if j in norm_dve:
                        nc.vector.tensor_scalar(oj, ps_j,
                                                rs[:, j:j + 1], nmr[:, j:j + 1],
                                                op0=OP.mult, op1=OP.add)
                    else:
                        nc.scalar.activation(oj, ps_j, AF.Identity,
                                             bias=nmr[:, j:j + 1],
                                             scale=rs[:, j:j + 1])
                if d == nsl - 1:
                    # drain the tail eagerly
                    if h == 0:
                        nc.scalar.dma_start(out_d.ap()[d, :, 0:SS, :],
                                            out_s[:, 0:SS, :])
                    else:
                        o = state.pop(("o", d))
                        nc.sync.dma_start(out_d.ap()[d, :, SS:SS + 2, :],
                                          o[:, SS:SS + 2, :])
                        nc.scalar.dma_start(out_d.ap()[d, :, SS + 2:SLAB, :],
                                            o[:, SS + 2:SLAB, :])
                elif h == 1:
                    nc.scalar.dma_start(out_d.ap()[d], state.pop(("o", d))[:])

            def loads(d):
                if d >= nsl:
                    return
                if d >= WARM:
                    qT_s = sin_pool.tile([128, SLAB, 2 * N], bf16, tag="qT_s")
                    nc.gpsimd.dma_start(qT_s[:], qT_d.ap()[d - WARM])
                    state[("q", d)] = qT_s
                vp_s = sin_pool.tile([N, SLAB, C + 4], bf16, tag="vp_s")
                nc.sync.dma_start(vp_s[:], v_d.ap()[d])
                state[("v", d)] = vp_s

            nc.sync.dma_start(dw_sb0[:], dw0_d.ap()[:])
            # v(0) split 3 ways so the first convs' input lands sooner
            vp_s0 = sin_pool.tile([N, SLAB, C + 4], bf16, tag="vp_s")
            nc.sync.dma_start(vp_s0[:, 0:3, :], v_d.ap()[0, :, 0:3, :])
            nc.scalar.dma_start(vp_s0[:, 3:6, :], v_d.ap()[0, :, 3:6, :])
            nc.gpsimd.dma_start(vp_s0[:, 6:8, :], v_d.ap()[0, :, 6:8, :])
            state[("v", 0)] = vp_s0
            for wd in range(warm):
                eng = (nc.scalar, nc.gpsimd, nc.sync)[wd % 3]
                eng.dma_start(pwT_sb0[:, wd, :], pwT0_d.ap()[:, wd, :])
            loads(1)
            nc.sync.dma_start(w2pw_t[:], w2pw_d.ap()[:])
            nc.sync.dma_start(w2dw_t[:], w2dw_d.ap()[:])
            nc.sync.dma_start(bpw_t[:], bpw_d.ap()[:])
            nc.sync.dma_start(bdwb_t[:], bdwb_d.ap()[:])
            stage0(0)
            stage0(1)
            for s in range(nss):
                if s % 2 == 0:
                    loads(s // 2 + 2)
                if s + 2 < nss:
                    stage0(s + 2)
                stage1(s)
                if s >= 1:
                    stage2a(s - 1)
                if s >= 2:
                    stage2b(s - 2)
            stage2a(nss - 1)
            stage2b(nss - 2)
            stage2b(nss - 1)

    nc.compile()
    return nc


def _get_nc(nb: int):
    if nb not in _cache:
        _cache[nb] = _build(nb)
    return _cache[nb]


def _host_prep(query, value, W_wl, b_wl, ln_gamma, ln_beta, n_cores=NCORES):
    """Build per-core input maps (numpy only)."""
    import ml_dtypes
    bf = ml_dtypes.bfloat16
    f32 = np.float32
    Bf = query.shape[0]
    nb = Bf // n_cores
    nsl = nb // SLAB
    warm = min(WARM, nsl)

    # qT[b] : [128, 2*N] with qT[b][p, h*N + n] = query[b, n, 128*h + p]
    qT = (
        query.transpose(0, 2, 1)          # [B, C, N]
        .reshape(Bf, 2, 128, N)
        .transpose(0, 2, 1, 3)            # [B, 128, 2, N]
        .reshape(Bf, 128, 2 * N)
    )
    qTs = np.ascontiguousarray(
        qT.reshape(Bf // SLAB, SLAB, 128, 2 * N).transpose(0, 2, 1, 3)
    ).astype(bf)                          # [B/SLAB, 128, SLAB, 2N]

    # per batch: [r0-slot(0), 0, v0..v255, 0, dw2-slot(0)]
    vp = np.zeros((Bf, N, C + 4), f32)
    vp[:, :, 2:C + 2] = value
    vps = np.ascontiguousarray(
        vp.reshape(Bf // SLAB, SLAB, N, C + 4).transpose(0, 2, 1, 3)
    ).astype(bf)                          # [B/SLAB, N, SLAB, C+4]

    # W_wl [C, N+K]: pw cols K:, dw cols :K; split C into two 128-halves
    w2pw = np.ascontiguousarray(
        W_wl[:, K:].reshape(2, 128, N).transpose(1, 0, 2)).astype(bf)
    w2dw = np.ascontiguousarray(
        W_wl[:, :K].reshape(2, 128, K).transpose(1, 0, 2)).astype(bf)
    bpw = np.ascontiguousarray(b_wl[K:].reshape(N, 1)).astype(f32)
    bdwb = np.ascontiguousarray(
        np.broadcast_to(b_wl[:K], (N, SLAB, K))).astype(f32)

    W64 = W_wl.astype(np.float64)
    b64 = b_wl.astype(np.float64)
    in_maps = []
    for c in range(n_cores):
        # warm slab's dy on host: cuts kernel startup latency
        q0 = query[c * nb:c * nb + warm * SLAB].astype(np.float64)
        dy0 = np.einsum('bnc,ck->bnk', q0, W64) + b64      # [WARM*SLAB, N, N+K]
        dw0 = np.ascontiguousarray(
            dy0[:, :, :K].transpose(1, 0, 2)).astype(f32)  # [N, warm*SLAB, K]
        # prefill warm conv heads: r0 = dw0/dw1 -> col0, dw2 -> col C+3
        wslabs = vps[c * nsl:c * nsl + warm]
        dwh = dy0[:, :, :K]                        # [WARM*SLAB, N, K] f64
        r0h = (dwh[:, :, 0] / dwh[:, :, 1]).astype(f32)   # [WARM*SLAB, N]
        dw2h = dwh[:, :, 2].astype(f32)
        for ds in range(warm):
            for j in range(SLAB):
                wslabs[ds, :, j, 0] = r0h[ds * SLAB + j].astype(bf)
                wslabs[ds, :, j, C + 3] = dw2h[ds * SLAB + j].astype(bf)
        pwT0 = np.ascontiguousarray(np.stack([
            np.concatenate([dy0[s * SLAB + j, :, K:].T for j in range(SLAB)],
                           axis=1) for s in range(warm)], axis=1)).astype(bf)
        m = {
            "qT": qTs[c * nsl + warm:(c + 1) * nsl],
            "v": vps[c * nsl:(c + 1) * nsl],
            "w2pw": w2pw,
            "w2dw": w2dw,
            "bpw": bpw,
            "bdwb": bdwb,
            "dw0": dw0,
            "pwT0": pwT0,
        }
        in_maps.append(m)
    return in_maps, nb


def _gather(results, n_cores, nb, ln_gamma, ln_beta):
    outs = []
    for c in range(n_cores):
        o = np.asarray(results[c]["out"]).astype(np.float32)  # [nsl, N, SLAB, C]
        o = o.transpose(0, 2, 1, 3).reshape(nb, N, C)
        outs.append(o)
    full = np.concatenate(outs, axis=0)
    if not (np.all(ln_gamma == np.float32(1.0))
            and np.all(ln_beta == np.float32(0.0))):
        full = full * ln_gamma + ln_beta
    return np.ascontiguousarray(full).astype(np.float32)


def kernel(query, value, W_wl, b_wl, ln_gamma, ln_beta):
    from concourse import bass_utils

    in_maps, nb = _host_prep(query, value, W_wl, b_wl, ln_gamma, ln_beta)
    nc = _get_nc(nb)
    res = bass_utils.run_bass_kernel_spmd(
        nc, in_maps, core_ids=list(range(NCORES)))
    return _gather(res.results, NCORES, nb, ln_gamma, ln_beta)
